# revision 4
# baseline (speedup 1.0000x reference)
"""ConditionalFeedForward (MoE routing) Trainium2 kernel.

Strategy: expert-parallel across 8 NeuronCores (E == n_cores == 8).
Host gathers the tokens routed to each expert (T*TOPK = 1024 token-slots
total, ~128/expert), pads to a fixed capacity C, and core e computes

    out_e = (silu(xg_e @ w1[e].T) * (xg_e @ w3[e].T)) @ w2[e]

for its expert only.  Weights/activations are cast to float16 on the host
(halves HBM traffic, 1 cyc/row on the PE; measured end-to-end L2 relative
error ~4.5e-4); PSUM accumulation is fp32.  Host layouts are pre-packed so
every DMA is a dense, fully contiguous 128-partition block.

Device layout (per core, P = 128):
  xg  [P, DO, C]     xg[p, o, t]    = x_gathered[t, o*P+p]      (d on partitions)
  w13 [HT, P, 2, DO, P] w13[i,p,j,o,c] = w_j[i*P+c, o*P+p]       (j: w1, w3)
  w2  [HT, P, D]     w2[i, p, d]    = w2[i*P+p, d]
  y   [P, DO, C]     y[p, o, t]     = out[t, o*P+p]

Phase 1 (per h-tile i): h1T/h3T [h=P, t=C] = sum_o w13[i,:,j,o,:].T @ xg[:,o,:]
  then gT = silu(h1T) * h3T  -> f16 SBUF tile, kept resident.
Phase 2 (two half-passes of 4 PSUM banks): out[d-tile o] [d=P, t=C] =
  sum_i w2[i][:, o*P:(o+1)*P].T @ gT[i].
"""

import os
import numpy as np

T, TOPK, E, H, D = 512, 2, 8, 2816, 1024
NCORES = 8
P = 128
HT = H // P   # 22 h-tiles
DO = D // P   # 8 d-tiles

_NC_CACHE = {}      # capacity C -> compiled Bacc module
_W_CACHE = {}       # weight pack cache: fingerprint -> (w13_packed, w2_packed)
LAST_PROFILE = None  # BassKernelResults of the most recent run (for test harness)


def _build(C):
    import concourse.mybir as mybir
    import concourse.tile as tile
    from concourse import bacc

    f16 = mybir.dt.float16
    f32 = mybir.dt.float32
    ACT = mybir.ActivationFunctionType

    nc = bacc.Bacc("TRN2", target_bir_lowering=False, debug=False)
    xg = nc.dram_tensor("xg", [P, DO, C], f16, kind="ExternalInput")
    w13 = nc.dram_tensor("w13", [HT, P, 2, DO, P], f16, kind="ExternalInput")
    w2 = nc.dram_tensor("w2", [HT, P, D], f16, kind="ExternalInput")
    y = nc.dram_tensor("y", [P, DO, C], f32, kind="ExternalOutput")

    with tile.TileContext(nc) as tc:
        from contextlib import ExitStack
        with ExitStack() as ctx:
            xpool = ctx.enter_context(tc.tile_pool(name="x", bufs=1))
            wpool = ctx.enter_context(tc.tile_pool(name="w13", bufs=4))
            w2pool = ctx.enter_context(tc.tile_pool(name="w2", bufs=HT))
            gpool = ctx.enter_context(tc.tile_pool(name="g", bufs=HT))
            apool = ctx.enter_context(tc.tile_pool(name="act", bufs=3))
            opool = ctx.enter_context(tc.tile_pool(name="osb", bufs=1))

            xg_sb = xpool.tile([P, DO, C], f16)
            nc.sync.dma_start(xg_sb[:], xg[:])

            w2_tiles = []
            g_tiles = []
            # Phase 1: h1T/h3T per h-tile, fused silu*mul -> resident gT tiles
            with tc.tile_pool(name="ps13", bufs=2, space="PSUM") as ps13:
                for i in range(HT):
                    w13_sb = wpool.tile([P, 2, DO, P], f16)
                    nc.sync.dma_start(w13_sb[:], w13[i])
                    # w2 streams on the other HWDGE ring; all tiles stay resident
                    w2_sb = w2pool.tile([P, D], f16)
                    nc.scalar.dma_start(w2_sb[:], w2[i])
                    w2_tiles.append(w2_sb)

                    ps1 = ps13.tile([P, C], f32)
                    ps3 = ps13.tile([P, C], f32)
                    for o in range(DO):
                        nc.tensor.matmul(ps1[:], w13_sb[:, 0, o, :], xg_sb[:, o, :],
                                         start=(o == 0), stop=(o == DO - 1))
                    for o in range(DO):
                        nc.tensor.matmul(ps3[:], w13_sb[:, 1, o, :], xg_sb[:, o, :],
                                         start=(o == 0), stop=(o == DO - 1))
                    # silu(h1) = h1 * sigmoid(h1)  (Silu LUT not in CoreSim; sigmoid is)
                    s1 = apool.tile([P, C], f32)
                    nc.scalar.activation(s1[:], ps1[:], ACT.Sigmoid)
                    t1 = apool.tile([P, C], f32, name="t1")
                    nc.vector.tensor_mul(t1[:], s1[:], ps1[:])
                    g_sb = gpool.tile([P, C], f16)
                    nc.vector.tensor_mul(g_sb[:], t1[:], ps3[:])
                    g_tiles.append(g_sb)

            # Phase 2: out[d-tile] = sum_i w2[i][:, d-slice].T @ gT[i]
            # Two half-passes of 4 accumulators so PSUM never exceeds 8 banks.
            out_sb = opool.tile([P, DO, C], f32)
            for half in range(2):
                with tc.tile_pool(name=f"pso{half}", bufs=DO // 2, space="PSUM") as pso:
                    outs = [pso.tile([P, C], f32, name="outp", tag="outp") for _ in range(DO // 2)]
                    for i in range(HT):
                        for j in range(DO // 2):
                            o = half * (DO // 2) + j
                            nc.tensor.matmul(outs[j][:],
                                             w2_tiles[i][:, o * P:(o + 1) * P],
                                             g_tiles[i][:],
                                             start=(i == 0), stop=(i == HT - 1))
                    for j in range(DO // 2):
                        o = half * (DO // 2) + j
                        nc.vector.tensor_copy(out_sb[:, o, :], outs[j][:])
            nc.gpsimd.dma_start(y[:], out_sb[:])

    nc.compile()
    return nc


def _fingerprint(*arrs):
    h = 0
    for a in arrs:
        v = a.reshape(-1)
        n = v.shape[0]
        step = max(1, n // 1024)
        sample = np.ascontiguousarray(v[:: step][:1024]).view(np.uint8)
        h ^= hash((a.shape, a.dtype.str, sample.tobytes(), id(a)))
    return h


def _pack_weights(w1, w2, w3):
    key = _fingerprint(w1, w2, w3)
    hit = _W_CACHE.get(key)
    if hit is not None:
        return hit
    w13p, w2p = [], []
    for e in range(E):
        a1 = w1[e].reshape(HT, P, DO, P).transpose(0, 3, 2, 1)  # [i, d_in, o, h_in]
        a3 = w3[e].reshape(HT, P, DO, P).transpose(0, 3, 2, 1)
        w13p.append(np.ascontiguousarray(
            np.stack([a1, a3], axis=2)).astype(np.float16))     # [i, p, 2, o, c]
        w2p.append(w2[e].reshape(HT, P, D).astype(np.float16))
    _W_CACHE.clear()
    _W_CACHE[key] = (w13p, w2p)
    return w13p, w2p


def kernel(x, expert_indices, w1, w2, w3):
    global LAST_PROFILE
    from concourse.bass_utils import run_bass_kernel_spmd

    x = np.asarray(x, dtype=np.float32)
    idx = np.asarray(expert_indices).astype(np.int64)
    w1 = np.asarray(w1, dtype=np.float32)
    w2 = np.asarray(w2, dtype=np.float32)
    w3 = np.asarray(w3, dtype=np.float32)

    # ---- host routing: slot s = t*TOPK + k -> expert idx.flat[s]
    flat_e = idx.reshape(-1)
    order = np.argsort(flat_e, kind="stable")
    counts = np.bincount(flat_e, minlength=E)
    starts = np.concatenate([[0], np.cumsum(counts)])
    C = max(144, int(-(-counts.max() // 16) * 16))

    nc = _NC_CACHE.get(C)
    if nc is None:
        nc = _NC_CACHE.setdefault(C, _build(C))

    w13p, w2p = _pack_weights(w1, w2, w3)
    x16 = x.astype(np.float16)

    in_maps = []
    slot_lists = []
    for e in range(E):
        slots = order[starts[e]:starts[e + 1]]
        slot_lists.append(slots)
        toks = slots // TOPK
        xg = np.zeros((C, D), np.float16)
        xg[: len(toks)] = x16[toks]
        xgp = np.ascontiguousarray(xg.T.reshape(DO, P, C).transpose(1, 0, 2))
        in_maps.append({"xg": xgp, "w13": w13p[e], "w2": w2p[e]})

    res = run_bass_kernel_spmd(nc, in_maps, core_ids=list(range(NCORES)))
    LAST_PROFILE = res

    out = np.zeros((T * TOPK, D), np.float32)
    for e in range(E):
        ye = res.results[e]["y"]                      # [P, DO, C]
        full = ye.transpose(2, 1, 0).reshape(C, D)    # [t, d]
        slots = slot_lists[e]
        out[slots] = full[: len(slots)]
    return out.reshape(T, TOPK, D)


# revision 6
# speedup vs baseline: 1.0868x; 1.0868x over previous
"""ConditionalFeedForward (MoE routing) Trainium2 kernel.

Strategy: expert-parallel across 8 NeuronCores (E == n_cores == 8).
Host gathers the tokens routed to each expert (T*TOPK = 1024 token-slots
total, ~128/expert), pads to a fixed capacity C, and core e computes

    out_e = (silu(xg_e @ w1[e].T) * (xg_e @ w3[e].T)) @ w2[e]

for its expert only.  Weights/activations are cast to float16 on the host
(halves HBM traffic, 1 cyc/row on the PE; measured end-to-end L2 relative
error ~4.5e-4); PSUM accumulation is fp32.  Host layouts are pre-packed so
every DMA is a dense, fully contiguous 128-partition block.

Device layout (per core, P = 128):
  xg  [P, DO, C]     xg[p, o, t]    = x_gathered[t, o*P+p]      (d on partitions)
  w13 [HT, P, 2, DO, P] w13[i,p,j,o,c] = w_j[i*P+c, o*P+p]       (j: w1, w3)
  w2  [HT, P, D]     w2[i, p, d]    = w2[i*P+p, d]
  y   [P, DO, C]     y[p, o, t]     = out[t, o*P+p]

Phase 1 (per h-tile i): h1T/h3T [h=P, t=C] = sum_o w13[i,:,j,o,:].T @ xg[:,o,:]
  then gT = silu(h1T) * h3T  -> f16 SBUF tile, kept resident.
Phase 2 (two half-passes of 4 PSUM banks): out[d-tile o] [d=P, t=C] =
  sum_i w2[i][:, o*P:(o+1)*P].T @ gT[i].
"""

import os
import numpy as np

T, TOPK, E, H, D = 512, 2, 8, 2816, 1024
NCORES = 8
P = 128
HT = H // P   # 22 h-tiles
DO = D // P   # 8 d-tiles

_NC_CACHE = {}      # capacity C -> compiled Bacc module
_W_CACHE = {}       # weight pack cache: fingerprint -> (w13_packed, w2_packed)
LAST_PROFILE = None  # BassKernelResults of the most recent run (for test harness)


def _build(C):
    import concourse.mybir as mybir
    import concourse.tile as tile
    from concourse import bacc

    f16 = mybir.dt.float16
    f32 = mybir.dt.float32
    ACT = mybir.ActivationFunctionType

    nc = bacc.Bacc("TRN2", target_bir_lowering=False, debug=False)
    xg = nc.dram_tensor("xg", [P, DO, C], f16, kind="ExternalInput")
    w13 = nc.dram_tensor("w13", [HT, P, 2, DO, P], f16, kind="ExternalInput")
    w2 = nc.dram_tensor("w2", [HT, P, D], f16, kind="ExternalInput")
    y = nc.dram_tensor("y", [P, DO, C], f32, kind="ExternalOutput")

    with tile.TileContext(nc) as tc:
        from contextlib import ExitStack
        with ExitStack() as ctx:
            xpool = ctx.enter_context(tc.tile_pool(name="x", bufs=1))
            wpool = ctx.enter_context(tc.tile_pool(name="w13", bufs=8))
            w2pool = ctx.enter_context(tc.tile_pool(name="w2", bufs=HT))
            gpool = ctx.enter_context(tc.tile_pool(name="g", bufs=HT))
            apool = ctx.enter_context(tc.tile_pool(name="act", bufs=3))
            opool = ctx.enter_context(tc.tile_pool(name="osb", bufs=1))

            xg_sb = xpool.tile([P, DO, C], f16)
            nc.sync.dma_start(xg_sb[:], xg[:])

            w2_tiles = []
            g_tiles = []
            # Phase 1: h1T/h3T per h-tile, fused silu*mul -> resident gT tiles.
            # All bulk weight DMAs share the single SP HWDGE ring so they
            # drain strictly in issue order: every w13 tile (which gates
            # phase-1 PE progress) lands before any w2 tile.  w2 DMAs are
            # emitted after the phase-1 loop and stream while phase 2A runs.
            with tc.tile_pool(name="ps13", bufs=2, space="PSUM") as ps13:
                for i in range(HT):
                    w13_sb = wpool.tile([P, 2, DO, P], f16)
                    nc.sync.dma_start(w13_sb[:], w13[i])
                    ps1 = ps13.tile([P, C], f32)
                    ps3 = ps13.tile([P, C], f32)
                    for o in range(DO):
                        nc.tensor.matmul(ps1[:], w13_sb[:, 0, o, :], xg_sb[:, o, :],
                                         start=(o == 0), stop=(o == DO - 1))
                    for o in range(DO):
                        nc.tensor.matmul(ps3[:], w13_sb[:, 1, o, :], xg_sb[:, o, :],
                                         start=(o == 0), stop=(o == DO - 1))
                    # silu(h1) = h1 * sigmoid(h1)  (Silu LUT not in CoreSim; sigmoid is)
                    s1 = apool.tile([P, C], f32)
                    nc.scalar.activation(s1[:], ps1[:], ACT.Sigmoid)
                    t1 = apool.tile([P, C], f32, name="t1")
                    nc.vector.tensor_mul(t1[:], s1[:], ps1[:])
                    g_sb = gpool.tile([P, C], f16)
                    nc.vector.tensor_mul(g_sb[:], t1[:], ps3[:])
                    g_tiles.append(g_sb)

            # w2 stream: same SP ring, queued behind all w13 tiles.
            for i in range(HT):
                w2_sb = w2pool.tile([P, D], f16, name="w2_sb")
                nc.sync.dma_start(w2_sb[:], w2[i])
                w2_tiles.append(w2_sb)

            # Phase 2: out[d-tile] = sum_i w2[i][:, d-slice].T @ gT[i]
            # Two half-passes of 4 accumulators so PSUM never exceeds 8 banks;
            # each half's output DMA (SWDGE ring) overlaps the other half.
            out_sb = opool.tile([P, DO, C], f32)
            for half in range(2):
                with tc.tile_pool(name=f"pso{half}", bufs=DO // 2, space="PSUM") as pso:
                    outs = [pso.tile([P, C], f32, name="outp", tag="outp") for _ in range(DO // 2)]
                    for i in range(HT):
                        for j in range(DO // 2):
                            o = half * (DO // 2) + j
                            nc.tensor.matmul(outs[j][:],
                                             w2_tiles[i][:, o * P:(o + 1) * P],
                                             g_tiles[i][:],
                                             start=(i == 0), stop=(i == HT - 1))
                    for j in range(DO // 2):
                        o = half * (DO // 2) + j
                        nc.vector.tensor_copy(out_sb[:, o, :], outs[j][:])
                lo, hi = half * (DO // 2), (half + 1) * (DO // 2)
                nc.gpsimd.dma_start(y[:, lo:hi, :], out_sb[:, lo:hi, :])

    nc.compile()
    return nc


def _fingerprint(*arrs):
    h = 0
    for a in arrs:
        v = a.reshape(-1)
        n = v.shape[0]
        step = max(1, n // 1024)
        sample = np.ascontiguousarray(v[:: step][:1024]).view(np.uint8)
        h ^= hash((a.shape, a.dtype.str, sample.tobytes(), id(a)))
    return h


def _pack_weights(w1, w2, w3):
    key = _fingerprint(w1, w2, w3)
    hit = _W_CACHE.get(key)
    if hit is not None:
        return hit
    w13p, w2p = [], []
    for e in range(E):
        a1 = w1[e].reshape(HT, P, DO, P).transpose(0, 3, 2, 1)  # [i, d_in, o, h_in]
        a3 = w3[e].reshape(HT, P, DO, P).transpose(0, 3, 2, 1)
        w13p.append(np.ascontiguousarray(
            np.stack([a1, a3], axis=2)).astype(np.float16))     # [i, p, 2, o, c]
        w2p.append(w2[e].reshape(HT, P, D).astype(np.float16))
    _W_CACHE.clear()
    _W_CACHE[key] = (w13p, w2p)
    return w13p, w2p


def kernel(x, expert_indices, w1, w2, w3):
    global LAST_PROFILE
    from concourse.bass_utils import run_bass_kernel_spmd

    x = np.asarray(x, dtype=np.float32)
    idx = np.asarray(expert_indices).astype(np.int64)
    w1 = np.asarray(w1, dtype=np.float32)
    w2 = np.asarray(w2, dtype=np.float32)
    w3 = np.asarray(w3, dtype=np.float32)

    # ---- host routing: slot s = t*TOPK + k -> expert idx.flat[s]
    flat_e = idx.reshape(-1)
    order = np.argsort(flat_e, kind="stable")
    counts = np.bincount(flat_e, minlength=E)
    starts = np.concatenate([[0], np.cumsum(counts)])
    C = max(144, int(-(-counts.max() // 16) * 16))

    nc = _NC_CACHE.get(C)
    if nc is None:
        nc = _NC_CACHE.setdefault(C, _build(C))

    w13p, w2p = _pack_weights(w1, w2, w3)
    x16 = x.astype(np.float16)

    in_maps = []
    slot_lists = []
    for e in range(E):
        slots = order[starts[e]:starts[e + 1]]
        slot_lists.append(slots)
        toks = slots // TOPK
        xg = np.zeros((C, D), np.float16)
        xg[: len(toks)] = x16[toks]
        xgp = np.ascontiguousarray(xg.T.reshape(DO, P, C).transpose(1, 0, 2))
        in_maps.append({"xg": xgp, "w13": w13p[e], "w2": w2p[e]})

    res = run_bass_kernel_spmd(nc, in_maps, core_ids=list(range(NCORES)))
    LAST_PROFILE = res

    out = np.zeros((T * TOPK, D), np.float32)
    for e in range(E):
        ye = res.results[e]["y"]                      # [P, DO, C]
        full = ye.transpose(2, 1, 0).reshape(C, D)    # [t, d]
        slots = slot_lists[e]
        out[slots] = full[: len(slots)]
    return out.reshape(T, TOPK, D)


# revision 11
# speedup vs baseline: 1.1563x; 1.0640x over previous
"""ConditionalFeedForward (MoE routing) Trainium2 kernel.

Strategy: expert-parallel across 8 NeuronCores (E == n_cores == 8).
Host gathers the tokens routed to each expert (T*TOPK = 1024 token-slots
total, ~128/expert), pads to a fixed capacity C, and core e computes

    out_e = (silu(xg_e @ w1[e].T) * (xg_e @ w3[e].T)) @ w2[e]

for its expert only.  Weights/activations are cast to float16 on the host
(halves HBM traffic, 1 cyc/row on the PE; measured end-to-end L2 relative
error ~4.5e-4); PSUM accumulation is fp32.  Host layouts are pre-packed so
every DMA is a dense, fully contiguous 128-partition block.

Device layout (per core, P = 128):
  xg  [P, DO, C]     xg[p, o, t]    = x_gathered[t, o*P+p]      (d on partitions)
  w13 [HT, P, 2, DO, P] w13[i,p,j,o,c] = w_j[i*P+c, o*P+p]       (j: w1, w3)
  w2  [HT, P, D]     w2[i, p, d]    = w2[i*P+p, d]
  y   [P, DO, C]     y[p, o, t]     = out[t, o*P+p]

Phase 1 (per h-tile i): h1T/h3T [h=P, t=C] = sum_o w13[i,:,j,o,:].T @ xg[:,o,:]
  then gT = silu(h1T) * h3T  -> f16 SBUF tile, kept resident.
Phase 2 (two half-passes of 4 PSUM banks): out[d-tile o] [d=P, t=C] =
  sum_i w2[i][:, o*P:(o+1)*P].T @ gT[i].
"""

import os
import numpy as np

T, TOPK, E, H, D = 512, 2, 8, 2816, 1024
NCORES = 8
P = 128
HT = H // P   # 22 h-tiles
DO = D // P   # 8 d-tiles

_NC_CACHE = {}      # capacity C -> compiled Bacc module
_W_CACHE = {}       # weight pack cache: fingerprint -> (w13_packed, w2_packed)
LAST_PROFILE = None  # BassKernelResults of the most recent run (for test harness)


def _build(C):
    import concourse.mybir as mybir
    import concourse.tile as tile
    from concourse import bacc

    f16 = mybir.dt.float16
    f32 = mybir.dt.float32
    ACT = mybir.ActivationFunctionType

    nc = bacc.Bacc("TRN2", target_bir_lowering=False, debug=False)
    xg = nc.dram_tensor("xg", [P, DO, C], f16, kind="ExternalInput")
    w13 = nc.dram_tensor("w13", [HT, P, 2, DO, P], f16, kind="ExternalInput")
    w2 = nc.dram_tensor("w2", [HT, P, D], f16, kind="ExternalInput")
    y = nc.dram_tensor("y", [P, DO, C], f32, kind="ExternalOutput")

    with tile.TileContext(nc) as tc:
        from contextlib import ExitStack
        with ExitStack() as ctx:
            xpool = ctx.enter_context(tc.tile_pool(name="x", bufs=1))
            wpool = ctx.enter_context(tc.tile_pool(name="w13", bufs=8))
            w2pool = ctx.enter_context(tc.tile_pool(name="w2", bufs=HT))
            gpool = ctx.enter_context(tc.tile_pool(name="g", bufs=HT))
            apool = ctx.enter_context(tc.tile_pool(name="act", bufs=3))
            opool = ctx.enter_context(tc.tile_pool(name="osb", bufs=1))

            # xg rides the SWDGE ring so it doesn't head-block w13[0] on SP
            xg_sb = xpool.tile([P, DO, C], f16)
            nc.gpsimd.dma_start(xg_sb[:], xg[:])

            w2_tiles = []
            g_tiles = []
            # Phase 1: h1T/h3T per h-tile, fused silu*mul -> resident gT tiles.
            # All bulk weight DMAs share the single SP HWDGE ring so they
            # drain strictly in issue order: every w13 tile (which gates
            # phase-1 PE progress) lands before any w2 tile.  w2 DMAs are
            # emitted after the phase-1 loop and stream while phase 2A runs.
            with tc.tile_pool(name="ps13", bufs=2, space="PSUM") as ps13:
                for i in range(HT):
                    w13_sb = wpool.tile([P, 2, DO, P], f16)
                    nc.sync.dma_start(w13_sb[:], w13[i])
                    ps1 = ps13.tile([P, C], f32)
                    ps3 = ps13.tile([P, C], f32)
                    for o in range(DO):
                        nc.tensor.matmul(ps1[:], w13_sb[:, 0, o, :], xg_sb[:, o, :],
                                         start=(o == 0), stop=(o == DO - 1))
                    for o in range(DO):
                        nc.tensor.matmul(ps3[:], w13_sb[:, 1, o, :], xg_sb[:, o, :],
                                         start=(o == 0), stop=(o == DO - 1))
                    # silu(h1) = h1 * sigmoid(h1)  (Silu LUT not in CoreSim; sigmoid is)
                    s1 = apool.tile([P, C], f32)
                    nc.scalar.activation(s1[:], ps1[:], ACT.Sigmoid)
                    t1 = apool.tile([P, C], f32, name="t1")
                    nc.vector.tensor_mul(t1[:], s1[:], ps1[:])
                    g_sb = gpool.tile([P, C], f16)
                    nc.vector.tensor_mul(g_sb[:], t1[:], ps3[:])
                    g_tiles.append(g_sb)

            # w2 stream: same SP ring, queued behind all w13 tiles; pairs of
            # h-tiles per DMA (512 KB) to stay at full HBM rate.
            for i in range(0, HT, 2):
                g2 = min(2, HT - i)
                w2_sb = w2pool.tile([P, 2, D], f16, name="w2_sb")
                nc.sync.dma_start(w2_sb[:, :g2, :],
                                  w2[i:i + g2].rearrange("g p d -> p g d"))
                for k in range(g2):
                    w2_tiles.append(w2_sb[:, k, :])

            # Phase 2: out[d-tile o] = sum_i w2[i][:, d-slice].T @ gT[i]
            # Single pass, all 8 accumulators live (phase-1 PSUM pool closed).
            out_sb = opool.tile([P, DO, C], f32)
            with tc.tile_pool(name="pso", bufs=DO, space="PSUM") as pso:
                outs = [pso.tile([P, C], f32, name="outp", tag="outp") for _ in range(DO)]
                for i in range(HT):
                    for o in range(DO):
                        nc.tensor.matmul(outs[o][:],
                                         w2_tiles[i][:, o * P:(o + 1) * P],
                                         g_tiles[i][:],
                                         start=(i == 0), stop=(i == HT - 1))
                for o in range(DO):
                    nc.vector.tensor_copy(out_sb[:, o, :], outs[o][:])
            nc.gpsimd.dma_start(y[:], out_sb[:])

    nc.compile()
    return nc


def _fingerprint(*arrs):
    h = 0
    for a in arrs:
        v = a.reshape(-1)
        n = v.shape[0]
        step = max(1, n // 1024)
        sample = np.ascontiguousarray(v[:: step][:1024]).view(np.uint8)
        h ^= hash((a.shape, a.dtype.str, sample.tobytes(), id(a)))
    return h


def _pack_weights(w1, w2, w3):
    key = _fingerprint(w1, w2, w3)
    hit = _W_CACHE.get(key)
    if hit is not None:
        return hit
    w13p, w2p = [], []
    for e in range(E):
        a1 = w1[e].reshape(HT, P, DO, P).transpose(0, 3, 2, 1)  # [i, d_in, o, h_in]
        a3 = w3[e].reshape(HT, P, DO, P).transpose(0, 3, 2, 1)
        w13p.append(np.ascontiguousarray(
            np.stack([a1, a3], axis=2)).astype(np.float16))     # [i, p, 2, o, c]
        w2p.append(w2[e].reshape(HT, P, D).astype(np.float16))
    _W_CACHE.clear()
    _W_CACHE[key] = (w13p, w2p)
    return w13p, w2p


def kernel(x, expert_indices, w1, w2, w3):
    global LAST_PROFILE
    from concourse.bass_utils import run_bass_kernel_spmd

    x = np.asarray(x, dtype=np.float32)
    idx = np.asarray(expert_indices).astype(np.int64)
    w1 = np.asarray(w1, dtype=np.float32)
    w2 = np.asarray(w2, dtype=np.float32)
    w3 = np.asarray(w3, dtype=np.float32)

    # ---- host routing: slot s = t*TOPK + k -> expert idx.flat[s]
    flat_e = idx.reshape(-1)
    order = np.argsort(flat_e, kind="stable")
    counts = np.bincount(flat_e, minlength=E)
    starts = np.concatenate([[0], np.cumsum(counts)])
    C = max(144, int(-(-counts.max() // 16) * 16))

    nc = _NC_CACHE.get(C)
    if nc is None:
        nc = _NC_CACHE.setdefault(C, _build(C))

    w13p, w2p = _pack_weights(w1, w2, w3)
    x16 = x.astype(np.float16)

    in_maps = []
    slot_lists = []
    for e in range(E):
        slots = order[starts[e]:starts[e + 1]]
        slot_lists.append(slots)
        toks = slots // TOPK
        xg = np.zeros((C, D), np.float16)
        xg[: len(toks)] = x16[toks]
        xgp = np.ascontiguousarray(xg.T.reshape(DO, P, C).transpose(1, 0, 2))
        in_maps.append({"xg": xgp, "w13": w13p[e], "w2": w2p[e]})

    res = run_bass_kernel_spmd(nc, in_maps, core_ids=list(range(NCORES)))
    LAST_PROFILE = res

    out = np.zeros((T * TOPK, D), np.float32)
    for e in range(E):
        ye = np.asarray(res.results[e]["y"], dtype=np.float32)  # [P, DO, C]
        full = ye.transpose(2, 1, 0).reshape(C, D)              # [t, d]
        slots = slot_lists[e]
        out[slots] = full[: len(slots)]
    return out.reshape(T, TOPK, D)
